# revision 2
# baseline (speedup 1.0000x reference)
"""Malvar demosaic on 8 trn2 NeuronCores — v2 (row-ordered outputs).

Input CFA [16,1,1024,1024] f32 + four 5x5 kernels -> output [16,3,1024,1024].

Pure data parallel (2 images/core), 9 bands of 124 output rows per image.

Measured-on-HW design points (axon pipelined-dispatch slope benches):
  - Store DMA throughput depends on engaged SBUF partitions AND on
    DRAM-address-sequential descriptors: ~300 GB/s for 124-128 contiguous
    4KB rows vs ~52 GB/s for the v1 parity/channel-interleaved stores.
    The matmul lhsT columns are therefore ROW-ORDERED (output partition =
    band row), making evictions partition-aligned and each channel store
    one DMA of contiguous rows.
  - fp32r matmuls are slower than bf16 on real HW; the entire conv stack
    runs in bf16 (lhsT weights exact in bf16; input rounding ~2^-9 rel).
  - Four 5x5 convs + passthrough fold into banded matmuls: per (channel,
    col-parity) an X-plane pass (vertical taps + identity), an S1 pass and
    an S2 pass on vector-precomputed horizontal pair sums (base18 scheme;
    18 matmuls per band accumulated over 6 PSUM banks).
"""

import numpy as np

import concourse.bass as bass
import concourse.mybir as mybir
import concourse.tile as tile
from concourse.bass_utils import run_bass_kernel_spmd

B, H, W = 16, 1024, 1024
N_CORES = 8
IMGS_PER_CORE = B // N_CORES
BAND = 124
NBANDS = (H + BAND - 1) // BAND
M = 124

# source per (channel, row-parity, col-parity): conv index 0..3 or "X"
_SEL = {
    (0, 0, 0): "X", (0, 0, 1): 1, (0, 1, 0): 2, (0, 1, 1): 3,   # R
    (1, 0, 0): 0, (1, 0, 1): "X", (1, 1, 0): "X", (1, 1, 1): 0,  # G
    (2, 0, 0): 3, (2, 0, 1): 2, (2, 1, 0): 1, (2, 1, 1): "X",    # B
}

# engine assignment
STORE_ENGS = ("sync", "scalar", "scalar")   # per output channel
LOAD_ENGS = ("gpsimd", "gpsimd")            # per row parity
CONV_ENG = "vector"                          # f32 -> bf16 conversion
EVICT_ENGS = ("vector", "scalar", "vector", "scalar", "vector", "scalar")
SCHEME = "base18"                            # "shift24" | "base18"


def _build_matrices(k5s):
    """Packed lhsT [128, 18*M] bf16; per (ch, cp): X / S1 / S2diag planes."""
    packed = np.zeros((128, 18 * M), dtype=np.float32)
    idx = 0
    for ch in range(3):
        for cp in range(2):
            Ms = [np.zeros((128, M), dtype=np.float32) for _ in range(3)]
            for d in range(BAND):
                m = d          # row-ordered outputs: lhsT column = band row
                src = _SEL[(ch, d % 2, cp)]
                if src == "X":
                    p = (d + 2) // 2 if d % 2 == 0 else 64 + (d + 1) // 2
                    Ms[0][p, m] += 1.0
                    continue
                k5 = k5s[src]
                assert np.allclose(k5[:, 1], k5[:, 3])
                assert np.allclose(k5[:, 0], k5[:, 4])
                assert np.allclose(k5[[0, 1, 3, 4], 0], 0.0)
                for dy in range(-2, 3):
                    r = d + dy
                    p = (r + 2) // 2 if r % 2 == 0 else 64 + (r + 1) // 2
                    Ms[0][p, m] += k5[2 + dy, 2]
                    Ms[1][p, m] += k5[2 + dy, 1]
                    if SCHEME == "base18":
                        Ms[2][p, m] += k5[2 + dy, 0]
                if SCHEME != "base18":
                    p0 = (d + 2) // 2 if d % 2 == 0 else 64 + (d + 1) // 2
                    Ms[2][p0, m] += k5[2, 0]
            for pl in range(3):
                packed[:, idx * M:(idx + 1) * M] = Ms[pl]
                idx += 1
    import ml_dtypes
    return packed.astype(ml_dtypes.bfloat16)


def _split_waits(nc, max_waits=1):
    """Walrus rejects >1 sem wait per instruction; hoist extras onto NoOps."""
    total = 0
    for bb in nc.main_func.blocks:
        insts = bb.bb.instructions if hasattr(bb, "bb") else bb.instructions
        i = 0
        while i < len(insts):
            ins = insts[i]
            si = ins.sync_info
            if si is not None and si.on_wait and len(si.on_wait) > max_waits:
                waits = list(si.on_wait)
                keep, hoist = waits[-max_waits:], waits[:-max_waits]
                nops = []
                for w in hoist:
                    nop = mybir.InstNoOp(
                        name=nc.get_next_instruction_name(),
                        engine=ins.engine, ins=[], outs=[],
                        sync_info=mybir.SyncInfo(on_wait=[w], on_update=[]))
                    nc.register_instruction(nop)
                    nops.append(nop)
                ins.sync_info = mybir.SyncInfo(
                    on_wait=keep, on_update=list(si.on_update or []))
                insts[i:i] = nops
                i += len(nops)
                total += len(nops)
            i += 1
    return total


REPEAT = 1
BUFS = 8


def _build_nc():
    bf16 = mybir.dt.bfloat16
    f32 = mybir.dt.float32
    nc = bass.Bass(target_bir_lowering=False, trn_type="TRN2")
    x = nc.dram_tensor("x", [IMGS_PER_CORE, 1, H, W], mybir.dt.float32r,
                       kind="ExternalInput")
    wts = nc.dram_tensor("wm", [128, 18 * M], bf16, kind="ExternalInput")
    zpad = nc.dram_tensor("zpad", [1, W], mybir.dt.float32r,
                          kind="ExternalInput")
    out = nc.dram_tensor("out", [IMGS_PER_CORE, 3, H, W], f32,
                         kind="ExternalOutput")

    def eng(name):
        return getattr(nc, name)

    with tile.TileContext(nc) as tc:
        with (
            tc.tile_pool(name="wpool", bufs=1) as wpool,
            tc.tile_pool(name="xpool", bufs=BUFS) as xpool,
            tc.tile_pool(name="spool", bufs=BUFS) as spool,
            tc.tile_pool(name="opool", bufs=BUFS) as opool,
            tc.tile_pool(name="psum", bufs=1, space="PSUM") as pspool,
        ):
            wt = wpool.tile([128, 18 * M], bf16)
            nc.gpsimd.dma_start(wt[:], wts[:])

            for _rep, b in ((r_, b_) for r_ in range(REPEAT)
                            for b_ in range(IMGS_PER_CORE)):
                for t in range(NBANDS):
                    r0 = t * BAND
                    n_rows = min(BAND, H - r0)

                    xt = xpool.tile([128, W + 4], mybir.dt.float32r, tag="x")
                    for par in range(2):
                        lo, hi = r0 - 2 + par, r0 + BAND + par + 1
                        vlo = lo if lo >= 0 else lo + 2
                        vhi = min(hi, H)
                        p0 = par * 64 + (vlo - lo) // 2
                        cnt = (vhi - vlo + 1) // 2
                        eng(LOAD_ENGS[par]).dma_start(
                            xt[p0:p0 + cnt, 2:W + 2], x[b, 0, vlo:vhi:2, :])
                        if lo < 0:
                            nc.gpsimd.dma_start(
                                xt[par * 64:par * 64 + 1, 2:W + 2],
                                zpad[:, :])
                        if hi > H:
                            nc.gpsimd.dma_start(
                                xt[p0 + cnt:p0 + cnt + 1, 2:W + 2],
                                zpad[:, :])
                    nc.gpsimd.memset(xt[:, 0:2].bitcast(f32), 0.0)
                    nc.gpsimd.memset(xt[:, W + 2:W + 4].bitcast(f32), 0.0)

                    xb = spool.tile([128, W + 4], bf16, tag="xb")
                    eng(CONV_ENG).tensor_copy(xb[:], xt[:])

                    s1 = spool.tile([128, W], bf16, tag="s1")
                    nc.vector.tensor_tensor(s1[:], xb[:, 1:W + 1],
                                            xb[:, 3:W + 3],
                                            mybir.AluOpType.add)
                    if SCHEME == "base18":
                        s2 = spool.tile([128, W], bf16, tag="s2")
                        nc.vector.tensor_tensor(s2[:], xb[:, 0:W],
                                                xb[:, 4:W + 4],
                                                mybir.AluOpType.add)

                    plane = opool.tile([128, 3 * W], f32, tag="pl")

                    for ci, (ch, cp) in enumerate(
                            (c, p) for c in range(3) for p in range(2)):
                        ps = pspool.tile([M, 512], f32, tag=f"ps{ci}",
                                         name=f"ps{ci}")
                        if SCHEME == "base18":
                            passes = (
                                (ci * 3 + 0, xb[:, 2 + cp:2 + cp + W:2]),
                                (ci * 3 + 1, s1[:, cp:W:2]),
                                (ci * 3 + 2, s2[:, cp:W:2]),
                            )
                        else:
                            passes = (
                                (ci * 3 + 0, xb[:, 2 + cp:2 + cp + W:2]),
                                (ci * 3 + 1, s1[:, cp:W:2]),
                                (ci * 3 + 2, xb[:, cp:cp + W:2]),
                                (ci * 3 + 2, xb[:, 4 + cp:W + 4:2]),
                            )
                        for pi, (wsl, rhs) in enumerate(passes):
                            nc.tensor.matmul(
                                ps[:], wt[:, wsl * M:(wsl + 1) * M], rhs,
                                start=(pi == 0), stop=(pi == len(passes) - 1))
                        dst = plane[0:M, ch * W + cp:ch * W + W:2]
                        ee = EVICT_ENGS[ci]
                        if ee == "scalar":
                            nc.scalar.copy(dst, ps[:])
                        else:
                            eng(ee).tensor_copy(dst, ps[:])

                    # one store per channel: row-ordered plane partitions
                    # -> DRAM-contiguous 4KB descriptors (~300 GB/s measured
                    # vs ~52 GB/s for parity/channel-interleaved patterns)
                    for ch in range(3):
                        eng(STORE_ENGS[ch]).dma_start(
                            out[b, ch, r0:r0 + n_rows, :],
                            plane[0:n_rows, ch * W:(ch + 1) * W])

    _split_waits(nc)
    nc.finalize()
    return nc


_CACHE = {}


def _get_nc():
    if "nc" not in _CACHE:
        _CACHE["nc"] = _build_nc()
    return _CACHE["nc"]


def kernel(CFA_inputs, GR_GB, Rg_RB_Bg_BR, Rg_BR_Bg_RB, Rb_BB_Br_RR,
           _trace=False):
    cfa = np.ascontiguousarray(np.asarray(CFA_inputs, dtype=np.float32))
    k5s = [np.asarray(k, dtype=np.float32)
           for k in (GR_GB, Rg_RB_Bg_BR, Rg_BR_Bg_RB, Rb_BB_Br_RR)]
    nc = _get_nc()

    wm = _build_matrices(k5s)
    zpad = np.zeros((1, W), dtype=np.float32)
    in_maps = [{"x": cfa[c * IMGS_PER_CORE:(c + 1) * IMGS_PER_CORE],
                "wm": wm, "zpad": zpad} for c in range(N_CORES)]

    res = run_bass_kernel_spmd(nc, in_maps, core_ids=list(range(N_CORES)),
                               trace=_trace)
    outs = np.concatenate([res.results[c]["out"] for c in range(N_CORES)],
                          axis=0)
    if _trace:
        kernel._last = res
    return outs
